# revision 31
# baseline (speedup 1.0000x reference)
"""Causal attention (flattened-head GQA variant) for TRN2, 8 NeuronCores.

Problem structure exploited:
  - K/V are group-projections tiled 4x along the head dim, and the score
    contraction runs over the full flattened 1024 dim.  Algebraically:
        att = Q @ tile(Kg,4)^T = (sum of Q's four 256-col blocks) @ Kg^T
        out = att_sm @ tile(Vg,4) = tile(att_sm @ Vg, 4)
    so the device only computes with 256-wide Qsum/Kg/Vg.
  - Projections run in fp8e4m3 DoubleRow mode (PE contracts 256 rows/pass at
    0.5 cycles/row = 4x fp16 rate) with error compensation: host splits
    x ~ x1 + x2 and W ~ W1 + W2 (each fp8, residual split), device computes
    x1W1 + x1W2 + x2W1 in one PSUM group (12 DR matmuls vs 16 fp16-equiv
    passes).  Dropped x2W2 term ~2^-8 relative.  Operands are pre-scaled by
    powers of 2 (x: 32, W: 4096 / 1024 for the Q block-sum) to sit in e4m3's
    normal range; the PSUM scale (2^15 Q / 2^17 K,V) is absorbed by the fused
    scale+bias tensor_scalar for Q/K, and for V rides into vg where it
    cancels against the ones-column (8*2^17) in the rowsum normalization.
  - Softmax needs no max-subtraction (logits bounded ~60; exp fits fp32),
    so scores are computed directly in the transposed layout
    U^T[s,t] = exp(Kg @ Qsum^T) and fed straight into the AV matmul as the
    stationary operand -- no on-device transposes at all.
  - Block-causal skipping: s-tiles entirely above the diagonal are never
    computed; diagonal 128x256 blocks are masked with precomputed 0/1 tiles.
  - Fused chunk loop keeps PE saturated while DMA streams x chunks.

Precision: fp8-3-term projections (~2^-8 rel), score matmul fp16, exp/AV
path bf16.  End-to-end absmax rel error vs fp32 reference ~1.75e-2.

Sharding: data-parallel over batch B=8, one batch per core, no collectives.
"""

import os
import numpy as np
import ml_dtypes
from contextlib import ExitStack

import concourse.tile as tile
from concourse import bacc, mybir
from concourse.bass_utils import run_bass_kernel_spmd

B, T, D = 8, 2048, 1024
C = 256          # group width (N_QUERY_GROUPS * HEAD_SIZE)
P = 128
ND = D // P      # 8 contraction tiles for projections
NDP = ND // 2    # 4 DoubleRow contraction pairs
NS = T // P      # 16 s-tiles
JB = 256         # t-chunk width
NJB = T // JB    # 8
NCORES = 8

F32 = mybir.dt.float32
FP16 = mybir.dt.float16
BF16 = mybir.dt.bfloat16
FP8 = mybir.dt.float8e4
DR = mybir.MatmulPerfMode.DoubleRow

SX = 32.0        # x pre-scale into e4m3 range
SWQ = 1024.0     # wq_s pre-scale (block-summed W spans +-0.125)
SWK = 4096.0     # wk/wv pre-scale (+-1/32)
QSC = 1.0 / (SX * SWQ)   # PSUM -> Q descale
KSC = 1.0 / (SX * SWK)   # PSUM -> K descale
VSC = SX * SWK           # V path stays scaled; ones-col = 8*VSC


def _build():
    nc = bacc.Bacc("TRN2", target_bir_lowering=False, debug=False)
    x1T = nc.dram_tensor("x1T", [D, T], FP8, kind="ExternalInput").ap()
    x2T = nc.dram_tensor("x2T", [D, T], FP8, kind="ExternalInput").ap()
    # w1|w2 stacked side-by-side: 512B contiguous rows dodge the <512B
    # 2x DMA latency multiplier.
    wsrc = {}
    for _n in ("q", "k", "v"):
        wsrc[_n] = nc.dram_tensor(f"w{_n}", [D, 2 * C], FP8,
                                  kind="ExternalInput").ap()
    bqk = nc.dram_tensor("bqk", [P, 4], F32, kind="ExternalInput").ap()
    bvb = nc.dram_tensor("bvb", [P, C], FP16, kind="ExternalInput").ap()
    msk = nc.dram_tensor("msk", [P, 2, JB], mybir.dt.float8e4, kind="ExternalInput").ap()
    o = nc.dram_tensor("o", [T, C], F32, kind="ExternalOutput").ap()

    with tile.TileContext(nc) as tc, ExitStack() as ctx:
        cst = ctx.enter_context(tc.tile_pool(name="cst", bufs=1))
        big = ctx.enter_context(tc.tile_pool(name="big", bufs=1))
        up = ctx.enter_context(tc.tile_pool(name="up", bufs=4))
        outp = ctx.enter_context(tc.tile_pool(name="outp", bufs=3))
        pp = ctx.enter_context(tc.tile_pool(name="pp", bufs=2, space="PSUM"))
        pst = ctx.enter_context(tc.tile_pool(name="pst", bufs=4, space="PSUM"))
        pav = ctx.enter_context(tc.tile_pool(name="pav", bufs=2, space="PSUM"))

        bqk_t = cst.tile([P, 4], F32, tag="bqk")
        bvb_t = cst.tile([P, C], FP16, tag="bvb")
        msk_t = cst.tile([P, 2, JB], mybir.dt.float8e4, tag="msk")

        wr12 = {}
        wr = {}
        for _n in ("q", "k", "v"):
            wr12[_n] = cst.tile([P, ND, 2 * C], FP8, tag=f"w{_n}",
                                name=f"wr_{_n}")
            wr[_n, 1] = wr12[_n][:, :, 0:C]
            wr[_n, 2] = wr12[_n][:, :, C:2 * C]

        # PE warm-up: ~5us of matmuls on a zeroed scratch tile while the
        # first DMAs land -- fills the idle start window and finishes the
        # PE clock ramp (HAM) before real work arrives.
        wrm = cst.tile([P, C], FP16, tag="wrm")
        nc.gpsimd.memset(wrm[:], 0.0)
        for wi in range(23):
            ps_w = pp.tile([P, 2 * JB], F32, tag="pp", name=f"warm_{wi}")
            nc.tensor.matmul(ps_w[:, :C], wrm[:, :P], wrm[:], start=True, stop=True)

        xtr1 = big.tile([P, ND, T], FP8, tag="xtr1")
        xtr2 = big.tile([P, ND, T], FP8, tag="xtr2")
        qkT = {"q": big.tile([P, 2, T], FP16, tag="qsT", name="qsT"),
               "k": big.tile([P, 2, T], FP16, tag="ksT", name="ksT")}
        vg = big.tile([P, NS, C + 1], BF16, tag="vg")
        nc.vector.memset(vg[:, :, C:C + 1], 8.0 * VSC)

        uts = {}

        def do_st(J):
            # scores^T -> exp for t-block J
            jt = slice(J * JB, (J + 1) * JB)
            ut = up.tile([P, NS, JB], BF16, tag="ut", name=f"ut_{J}")
            uts[J] = ut
            for sp in range(J + 1):
                si0 = 2 * sp
                ps_t = pst.tile([P, 2 * JB], F32, tag="pst",
                                name=f"pst_{J}_{sp}")
                if sp < J:
                    for h in range(2):
                        si = si0 + h
                        for ct in range(2):
                            nc.tensor.matmul(
                                ps_t[:, h * JB:(h + 1) * JB],
                                qkT["k"][:, ct, si * P:(si + 1) * P],
                                qkT["q"][:, ct, jt],
                                start=(ct == 0), stop=(ct == 1),
                            )
                    nc.scalar.activation(ut[:, si0:si0 + 2, :], ps_t[:],
                                         mybir.ActivationFunctionType.Exp)
                else:
                    # diagonal pair: si0 needs all 256 t-cols; si0+1 only
                    # its second 128 (AV q=0 never reads si0+1) -> N=128
                    for ct in range(2):
                        nc.tensor.matmul(
                            ps_t[:, 0:JB],
                            qkT["k"][:, ct, si0 * P:(si0 + 1) * P],
                            qkT["q"][:, ct, jt],
                            start=(ct == 0), stop=(ct == 1),
                        )
                    for ct in range(2):
                        nc.tensor.matmul(
                            ps_t[:, JB:JB + P],
                            qkT["k"][:, ct, (si0 + 1) * P:(si0 + 2) * P],
                            qkT["q"][:, ct, J * JB + P:(J + 1) * JB],
                            start=(ct == 0), stop=(ct == 1),
                        )
                    nc.scalar.activation(ut[:, si0, :], ps_t[:, 0:JB],
                                         mybir.ActivationFunctionType.Exp)
                    nc.scalar.activation(ut[:, si0 + 1, P:JB],
                                         ps_t[:, JB:JB + P],
                                         mybir.ActivationFunctionType.Exp)
                    nc.vector.tensor_tensor(ut[:, si0, :], ut[:, si0, :],
                                            msk_t[:, 0, :],
                                            mybir.AluOpType.mult)
                    nc.vector.tensor_tensor(ut[:, si0 + 1, P:JB],
                                            ut[:, si0 + 1, P:JB],
                                            msk_t[:, 1, P:JB],
                                            mybir.AluOpType.mult)

        def dr_group(ps, mat, cols, psl, w_stationary=True):
            # 3-term compensated fp8 projection: x1W1 + x1W2 + x2W1.
            # w_stationary: True -> psum [c, t] (Q/K transposed layout);
            # False -> psum [t, c] (V layout).
            first = True
            for (xt, wh) in ((xtr1, 1), (xtr1, 2), (xtr2, 1)):
                for dp in range(NDP):
                    wap = wr[mat, wh][:, 2 * dp:2 * dp + 2, cols]
                    xap = xt[:, 2 * dp:2 * dp + 2, psl]
                    nc.tensor.matmul(
                        ps,
                        wap if w_stationary else xap,
                        xap if w_stationary else wap,
                        start=first, stop=(xt is xtr2 and dp == NDP - 1),
                        perf_mode=DR,
                    )
                    first = False

        def do_v(tb):
            # V projection for chunk tb's two s-tiles
            for si in (2 * tb, 2 * tb + 1):
                pv = pav.tile([P, C + 1], F32, tag="pav", name=f"pv_{si}")[:, :JB]
                dr_group(pv, "v", slice(0, C), slice(si * P, (si + 1) * P),
                         w_stationary=False)
                nc.vector.tensor_tensor(vg[:, si, :C], pv, bvb_t[:],
                                        mybir.AluOpType.add)

        def do_av(J):
            ut = uts[J]
            for q in range(2):
                tci = 2 * J + q
                if J == NJB - 1 and q == 1:
                    # Final tile: split into column halves so the hi half's
                    # recip/mul/DMA overlap the lo half's accumulation --
                    # shortens the exposed end-of-kernel output chain.
                    pa_ht = pp.tile([P, 2 * JB], F32, tag="pp", name="pav_fh")
                    pa_lt = pp.tile([P, 2 * JB], F32, tag="pp", name="pav_fl")
                    pa_hi, pa_lo = pa_ht[:, :P + 1], pa_lt[:, :P]
                    stat = ut[:, :, q * P:(q + 1) * P]
                    for si in range(tci + 1):
                        nc.tensor.matmul(pa_hi, stat[:, si], vg[:, si, P:],
                                         start=(si == 0), stop=(si == tci))
                    recip = outp.tile([P, 1], F32, tag="recip")
                    nc.vector.reciprocal(recip[:], pa_hi[:, P:P + 1])
                    ob_hi = outp.tile([P, C], F32, tag="ob", name="ob_hi")[:, :P]
                    nc.vector.tensor_scalar_mul(ob_hi, pa_hi[:, :P], recip[:])
                    nc.sync.dma_start(o[tci * P:(tci + 1) * P, P:], ob_hi)
                    for si in range(tci + 1):
                        nc.tensor.matmul(pa_lo, stat[:, si], vg[:, si, :P],
                                         start=(si == 0), stop=(si == tci))
                    ob_lo = outp.tile([P, C], F32, tag="ob", name="ob_lo")[:, :P]
                    nc.vector.tensor_scalar_mul(ob_lo, pa_lo, recip[:])
                    nc.sync.dma_start(o[tci * P:(tci + 1) * P, :P], ob_lo)
                    continue
                pa = pav.tile([P, C + 1], F32, tag="pav", name=f"pav_{J}_{q}")
                for si in range(tci + 1):
                    nc.tensor.matmul(
                        pa[:],
                        ut[:, si, q * P:(q + 1) * P],
                        vg[:, si, :],
                        start=(si == 0), stop=(si == tci),
                    )
                recip = outp.tile([P, 1], F32, tag="recip")
                nc.vector.reciprocal(recip[:], pa[:, C:C + 1])
                ob = outp.tile([P, C], F32, tag="ob")
                nc.vector.tensor_scalar_mul(ob[:], pa[:, :C], recip[:])
                nc.sync.dma_start(o[tci * P:(tci + 1) * P, :], ob[:])

        def do_proj(psl):
            pw = psl.stop - psl.start
            for ct in range(2):
                for mi, (mat, dsc) in enumerate((("q", QSC), ("k", KSC))):
                    ps_p = pp.tile([P, 2 * JB], F32, tag="pp",
                                   name=f"pp_{mat}{ct}_{psl.start}")[:, :pw]
                    dr_group(ps_p, mat, slice(ct * P, (ct + 1) * P), psl)
                    nc.vector.tensor_scalar(
                        qkT[mat][:, ct, psl],
                        ps_p,
                        dsc,
                        bqk_t[:, 2 * mi + ct: 2 * mi + ct + 1],
                        mybir.AluOpType.mult,
                        mybir.AluOpType.add,
                    )

        def proj_term_phase(psl, tiles, mats, xt, wh, pools=None,
                            close=False):
            # Startup building block: emit one (x, w-half) term (4 DR) for
            # each (mat, ct) group in `mats`.  With `pools`, groups are
            # opened (start=True); with `close`, groups are closed and the
            # fused scale+bias writes qkT.
            pw = psl.stop - psl.start
            for gi, (mat, ct) in enumerate(mats):
                if pools is not None:
                    pool, shape, tag = pools[gi]
                    ps_p = pool.tile(shape, F32, tag=tag,
                                     name=f"pps_{mat}{ct}_{psl.start}")[:, :pw]
                    tiles[mat, ct] = ps_p
                else:
                    ps_p = tiles[mat, ct]
                for dp in range(NDP):
                    nc.tensor.matmul(
                        ps_p,
                        wr[mat, wh][:, 2 * dp:2 * dp + 2,
                                    ct * P:(ct + 1) * P],
                        xt[:, 2 * dp:2 * dp + 2, psl],
                        start=(pools is not None and dp == 0),
                        stop=(close and dp == NDP - 1),
                        perf_mode=DR,
                    )
                if close:
                    mi, dsc = (0, QSC) if mat == "q" else (1, KSC)
                    nc.vector.tensor_scalar(
                        qkT[mat][:, ct, psl],
                        ps_p,
                        dsc,
                        bqk_t[:, 2 * mi + ct: 2 * mi + ct + 1],
                        mybir.AluOpType.mult,
                        mybir.AluOpType.add,
                    )

        # ---- startup DMA stream, ordered to match PE consumption: the
        # first real matmul needs wq+x1c0; x2/wk trail behind the x1
        # phases; wv slots in before x pairs 2-3 (V/AV for chunks 0-1 are
        # deferred past pair 1's projections). ----
        ts0 = slice(0, JB)
        ts1 = slice(JB, 2 * JB)
        p0 = slice(0, 2 * JB)
        x1r = x1T.rearrange("(o p) t -> p o t", p=P)
        x2r = x2T.rearrange("(o p) t -> p o t", p=P)
        wq_r = wsrc["q"].rearrange("(o p) c -> p o c", p=P)
        nc.sync.dma_start(wr12["q"][:, :, 0:C], wq_r[:, :, 0:C])
        nc.sync.dma_start(xtr1[:, :, p0], x1r[:, :, p0])
        nc.sync.dma_start(wr12["q"][:, :, C:2 * C], wq_r[:, :, C:2 * C])
        nc.sync.dma_start(wr12["k"][:],
                          wsrc["k"].rearrange("(o p) c -> p o c", p=P))
        nc.sync.dma_start(bqk_t[:], bqk)
        nc.sync.dma_start(xtr2[:, :, p0], x2r[:, :, p0])
        nc.sync.dma_start(msk_t[:], msk)
        nc.sync.dma_start(bvb_t[:], bvb)
        # Term-phased startup: (x1,w1) terms need only the first weight
        # half + x1; (x1,w2) and (x2,w1) phases follow as their DMAs land.
        # 8 held PSUM groups: 4 from pst, 2 from pp, 2 from pav.
        t_tiles = {}
        QG = (("q", 0), ("q", 1))
        KG = (("k", 0), ("k", 1))
        tile_src = {ts0.start: [(pst, [P, 2 * JB], "pst")] * 2,
                    ts1.start: [(pst, [P, 2 * JB], "pst")] * 2}
        tile_src2 = {ts0.start: [(pp, [P, 2 * JB], "pp")] * 2,
                     ts1.start: [(pav, [P, C + 1], "pav")] * 2}
        tiles = {ts0.start: {}, ts1.start: {}}
        for psl in (ts0, ts1):
            proj_term_phase(psl, tiles[psl.start], QG, xtr1, 1,
                            pools=tile_src[psl.start])
        for psl in (ts0, ts1):
            proj_term_phase(psl, tiles[psl.start], QG, xtr1, 2)
        for psl in (ts0, ts1):
            proj_term_phase(psl, tiles[psl.start], KG, xtr1, 1,
                            pools=tile_src2[psl.start])
        for psl in (ts0, ts1):
            proj_term_phase(psl, tiles[psl.start], KG, xtr1, 2)
        for psl in (ts0, ts1):
            proj_term_phase(psl, tiles[psl.start], QG + KG, xtr2, 1,
                            close=True)
        do_st(0)
        do_st(1)

        # ---- pair 1: x pair 1 queues ahead of the V weights; V/AV for
        # chunks 0-1 run after pair 1's projections and scores, by which
        # time wv has landed. ----
        pts = slice(2 * JB, 4 * JB)
        nc.sync.dma_start(xtr1[:, :, pts], x1r[:, :, pts])
        nc.sync.dma_start(xtr2[:, :, pts], x2r[:, :, pts])
        nc.sync.dma_start(wr12["v"][:],
                          wsrc["v"].rearrange("(o p) c -> p o c", p=P))
        # (wv last: V/AV for chunks 0-1 below are the first consumers)
        do_v(0)
        do_av(0)
        do_v(1)
        do_av(1)
        do_proj(pts)
        for tb in (2, 3):
            do_st(tb)
            do_v(tb)
            do_av(tb)

        # ---- pairs 2-3 ----
        for pb in (2, 3):
            tb0 = 2 * pb
            pts = slice(tb0 * JB, (tb0 + 2) * JB)
            nc.sync.dma_start(xtr1[:, :, pts], x1r[:, :, pts])
            nc.sync.dma_start(xtr2[:, :, pts], x2r[:, :, pts])
            do_proj(pts)
            for tb in (tb0, tb0 + 1):
                do_st(tb)
                do_v(tb)
                do_av(tb)

    nc.compile()
    return nc


_CACHE = {}
LAST_EXEC_TIME_NS = None


def _get_nc():
    if "nc" not in _CACHE:
        _CACHE["nc"] = _build()
    return _CACHE["nc"]


E4 = ml_dtypes.float8_e4m3


def _split8(a, scale):
    a1 = (a * scale).astype(E4)
    a2 = (a * scale - a1.astype(np.float32)).astype(E4)
    return a1, a2


def kernel(x, Wq, bq, Wk, bk, Wv, bv):
    x = np.asarray(x, dtype=np.float32)
    Wq = np.asarray(Wq, dtype=np.float32)
    bq = np.asarray(bq, dtype=np.float32)
    Wk = np.asarray(Wk, dtype=np.float32)
    bk = np.asarray(bk, dtype=np.float32)
    Wv = np.asarray(Wv, dtype=np.float32)
    bv = np.asarray(bv, dtype=np.float32)

    # Fold the 4x head-tiling into the weights: contraction with tile(Kg,4)
    # equals contraction of block-summed Q with Kg.
    wq_s = Wq.reshape(D, 4, C).sum(axis=1, dtype=np.float64).astype(np.float32)
    bq_s = bq.reshape(4, C).sum(axis=0, dtype=np.float64).astype(np.float32)

    bqk = np.stack([bq_s[:P], bq_s[P:], bk[:P], bk[P:]], axis=1).astype(np.float32)
    bvb = np.broadcast_to(VSC * bv, (P, C)).astype(np.float32)

    # Diagonal-block causal masks: keep t >= s  <=>  j >= 128*m + p.
    jj = np.arange(JB)[None, None, :]
    pp_ = np.arange(P)[:, None, None]
    mm = np.arange(2)[None, :, None]
    msk = (jj >= P * mm + pp_).astype(ml_dtypes.float8_e4m3)

    shared = {
        "wq": np.ascontiguousarray(np.concatenate(_split8(wq_s, SWQ), axis=1)),
        "wk": np.ascontiguousarray(np.concatenate(_split8(Wk, SWK), axis=1)),
        "wv": np.ascontiguousarray(np.concatenate(_split8(Wv, SWK), axis=1)),
        "bqk": np.ascontiguousarray(bqk),
        "bvb": np.ascontiguousarray(bvb.astype(np.float16)),
        "msk": np.ascontiguousarray(msk),
    }
    in_maps = []
    for b in range(B):
        x1, x2 = _split8(x[b].T, SX)
        m = dict(shared)
        m["x1T"] = np.ascontiguousarray(x1)
        m["x2T"] = np.ascontiguousarray(x2)
        in_maps.append(m)

    nc = _get_nc()
    try:
        res = run_bass_kernel_spmd(nc, in_maps, core_ids=list(range(NCORES)))
    except ModuleNotFoundError:
        # BASS_TRACE=1 requests NTFF profiling, but this container type has
        # no axon NTFF hook (antenv.axon_hooks absent) -- rerun untraced.
        os.environ["BASS_NEVER_TRACE"] = "1"
        res = run_bass_kernel_spmd(nc, in_maps, core_ids=list(range(NCORES)))
    global LAST_EXEC_TIME_NS
    LAST_EXEC_TIME_NS = res.exec_time_ns
    if res.exec_time_ns is not None:
        print(f"HW exec time: {res.exec_time_ns} ns")

    out = np.empty((1, B, T, 4 * C), dtype=np.float32)
    for b in range(B):
        ob = res.results[b]["o"]
        out[0, b] = np.tile(ob, (1, 4))
    return out
